# revision 16
# baseline (speedup 1.0000x reference)
"""Trainium2 Bass kernel for the Slayer module.

Math: y[b,o] = sum_i sum_k coef[o*128+i,k]*scale_sp[o*128+i] * sin((k+1)*x[b,i])
             + sum_i scale_base[o*128+i] * silu(x[b,i]) + bias_w[o]

Reformulated as 9 accumulating matmuls over folded weights:
  W[i, j, o] (j=0..7 sin bases, j=8 silu base), feats[i, j, b] computed on ACT.

Sharding: data-parallel on batch across 8 cores (128 rows each); weights
replicated.
"""

import numpy as np

import concourse.bass as bass
from concourse import bacc
import concourse.tile as tile
from concourse import mybir
from concourse.bass_utils import run_bass_kernel_spmd

IN_DIM = 128
OUT_DIM = 256
NUM_F = 8
BATCH = 1024
N_CORES = 8
B_SH = BATCH // N_CORES  # 128
NFUNC = NUM_F + 1  # 8 sin bases + 1 silu base

F32 = mybir.dt.float32
F32R = mybir.dt.float32r

_KERNEL_CACHE = {}


def _build(grid_vals):
    key = tuple(float(g) for g in grid_vals)
    if key in _KERNEL_CACHE:
        return _KERNEL_CACHE[key]

    nc = bacc.Bacc()
    xt = nc.declare_dram_parameter("xt", [IN_DIM, B_SH], F32, isOutput=False)
    # w slots: 0-7 sin bases, 8 silu base, 9 bias (replicated over i),
    # 10 ones/128 (bias-matmul lhsT); declared f32r so the DMA is the
    # "rounded" producer walrus wants for f32r matmul inputs.
    w = nc.declare_dram_parameter("w", [IN_DIM, NFUNC + 2, OUT_DIM], F32R, isOutput=False)
    y = nc.declare_dram_parameter("y", [B_SH, OUT_DIM], F32, isOutput=True)

    with tile.TileContext(nc) as tc:
        with (
            tc.tile_pool(name="sb", bufs=1) as sb,
            tc.tile_pool(name="ps", bufs=1, space="PSUM") as ps,
        ):
            # x shard, transposed to [i, b] so i (contraction) is on partitions
            sb_x = sb.tile([IN_DIM, B_SH], F32)
            nc.sync.dma_start(sb_x[:], xt[:])

            sb_w = sb.tile([IN_DIM, NFUNC + 2, OUT_DIM], F32R)
            nc.sync.dma_start(sb_w[:], w[:])

            # features. ACT's sin spline is only accurate on ~[-4.15, 4.15],
            # so range-reduce on DVE into [-pi, pi]:
            #   n = round(g*x / 2pi)  (via the 1.5*2^23 magic-add trick)
            #   r = g*x - 2pi*n
            # silu is accurate over at least +-12, no reduction needed.
            MAGIC = 12582912.0  # 1.5 * 2**23
            TWO_PI = 2.0 * np.pi
            t8 = sb.tile([IN_DIM, NUM_F, B_SH], F32)
            for j, g in enumerate(key):
                nc.vector.tensor_scalar(
                    t8[:, j, :],
                    sb_x[:],
                    float(g),
                    None,
                    mybir.AluOpType.mult,
                )
            tn = sb.tile([IN_DIM, NUM_F, B_SH], F32)
            nc.vector.tensor_scalar(
                tn[:], t8[:], 1.0 / TWO_PI, MAGIC,
                mybir.AluOpType.mult, mybir.AluOpType.add,
            )
            nc.vector.tensor_scalar(
                tn[:], tn[:], MAGIC, TWO_PI,
                mybir.AluOpType.subtract, mybir.AluOpType.mult,
            )
            nc.vector.tensor_tensor(
                t8[:], t8[:], tn[:], mybir.AluOpType.subtract
            )
            feats = sb.tile([IN_DIM, NFUNC, B_SH], F32R)
            nc.scalar.activation(
                feats[:, 0:NUM_F, :],
                t8[:],
                mybir.ActivationFunctionType.Sin,
            )
            nc.scalar.activation(
                feats[:, NUM_F, :], sb_x[:], mybir.ActivationFunctionType.Silu
            )

            # The self-loading f32r matmul ISA slot carries only ONE sync
            # wait; a matmul depending on both ACT (feats) and DMA (w) sems
            # fails codegen. Absorb the w DMA wait into a tiny dummy matmul
            # so the real matmuls only wait on ACT.
            dummy_ps = ps.tile([1, 4], F32, tag="dummy")
            nc.tensor.matmul(
                dummy_ps[:1, :1],
                lhsT=sb_w[:, 0, 0:1].bitcast(F32),
                rhs=sb_w[:, 0, 0:1].bitcast(F32),
                start=True,
                stop=True,
            )

            # accumulating matmuls: psum[b, o] += feats[:, j, :].T @ W[:, j, :]
            psum = ps.tile([B_SH, OUT_DIM], F32)
            for j in range(NFUNC):
                nc.tensor.matmul(
                    psum[:],
                    lhsT=feats[:, j, :],
                    rhs=sb_w[:, j, :],
                    start=(j == 0),
                    stop=False,
                )
            # bias: += ones/128 . bias_row  (both live in the w tensor)
            nc.tensor.matmul(
                psum[:],
                lhsT=sb_w[:, NFUNC + 1, :B_SH],
                rhs=sb_w[:, NFUNC, :],
                start=False,
                stop=True,
            )

            out_sb = sb.tile([B_SH, OUT_DIM], F32)
            nc.vector.tensor_copy(out_sb[:], psum[:])
            nc.sync.dma_start(y[:], out_sb[:])

    nc.finalize()
    _KERNEL_CACHE[key] = nc
    return nc


def _prep_inputs(x, grid, coef, scale_sp, scale_base, bias_w):
    x = np.asarray(x, dtype=np.float32)
    coef = np.asarray(coef, dtype=np.float32)
    scale_sp = np.asarray(scale_sp, dtype=np.float32)
    scale_base = np.asarray(scale_base, dtype=np.float32)
    bias_w = np.asarray(bias_w, dtype=np.float32).reshape(1, OUT_DIM)

    # fold scales into the per-basis weight matrices, laid out [i, j, o]
    coef3 = coef.reshape(OUT_DIM, IN_DIM, NUM_F)  # [o, i, k]
    ssp = scale_sp.reshape(OUT_DIM, IN_DIM)  # [o, i]
    sba = scale_base.reshape(OUT_DIM, IN_DIM)
    w = np.zeros((IN_DIM, NFUNC + 2, OUT_DIM), np.float32)
    w[:, :NUM_F, :] = np.transpose(coef3 * ssp[:, :, None], (1, 2, 0))
    w[:, NUM_F, :] = sba.T
    w[:, NFUNC, :] = bias_w[0][None, :]  # bias row, replicated over i
    w[:, NFUNC + 1, :B_SH] = 1.0 / IN_DIM  # ones/128 lhsT for bias matmul

    xt = np.ascontiguousarray(x.T)  # [i, b] full

    in_maps = []
    for c in range(N_CORES):
        in_maps.append(
            {
                "xt": np.ascontiguousarray(xt[:, c * B_SH : (c + 1) * B_SH]),
                "w": w,
            }
        )
    return in_maps


def _run(inputs, trace=False):
    nc = _build(np.asarray(inputs["grid"], dtype=np.float32))
    in_maps = _prep_inputs(
        inputs["x"],
        inputs["grid"],
        inputs["coef"],
        inputs["scale_sp"],
        inputs["scale_base"],
        inputs["bias_w"],
    )
    res = run_bass_kernel_spmd(nc, in_maps, list(range(N_CORES)), trace=trace)
    y = np.concatenate([res.results[c]["y"] for c in range(N_CORES)], axis=0)
    return y.astype(np.float32), res


def kernel(**inputs):
    y, _ = _run(inputs, trace=False)
    return y


# revision 17
# speedup vs baseline: 1.0475x; 1.0475x over previous
"""Trainium2 Bass kernel for the Slayer module.

Math: y[b,o] = sum_i sum_k coef[o*128+i,k]*scale_sp[o*128+i] * sin((k+1)*x[b,i])
             + sum_i scale_base[o*128+i] * silu(x[b,i]) + bias_w[o]

Reformulated as 9 accumulating matmuls over folded weights:
  W[i, j, o] (j=0..7 sin bases, j=8 silu base), feats[i, j, b] computed on ACT.

Sharding: data-parallel on batch across 8 cores (128 rows each); weights
replicated.
"""

import numpy as np

import concourse.bass as bass
from concourse import bacc
import concourse.tile as tile
from concourse import mybir
from concourse.bass_utils import run_bass_kernel_spmd

IN_DIM = 128
OUT_DIM = 256
NUM_F = 8
BATCH = 1024
N_CORES = 8
B_SH = BATCH // N_CORES  # 128
NFUNC = NUM_F + 1  # 8 sin bases + 1 silu base

F32 = mybir.dt.float32
F32R = mybir.dt.float32r

_KERNEL_CACHE = {}


def _build(grid_vals):
    key = tuple(float(g) for g in grid_vals)
    if key in _KERNEL_CACHE:
        return _KERNEL_CACHE[key]

    nc = bacc.Bacc()
    xt = nc.declare_dram_parameter("xt", [IN_DIM, B_SH], F32, isOutput=False)
    # w slots: 0-7 sin bases, 8 silu base, 9 bias row (replicated over i).
    # Declared f32r: the DMA is then the "rounded" producer walrus requires
    # for f32r matmul inputs.
    w = nc.declare_dram_parameter("w", [IN_DIM, 10, OUT_DIM], F32R, isOutput=False)
    y = nc.declare_dram_parameter("y", [B_SH, OUT_DIM], F32, isOutput=True)

    MAGIC = 12582912.0  # 1.5 * 2**23
    TWO_PI = 2.0 * np.pi
    H = NUM_F // 2  # 4 sin bases per pipeline half

    with tile.TileContext(nc) as tc:
        with (
            tc.tile_pool(name="sb", bufs=1) as sb,
            tc.tile_pool(name="ps", bufs=1, space="PSUM") as ps,
        ):
            # x shard, transposed to [i, b] so i (contraction) is on partitions
            sb_x = sb.tile([IN_DIM, B_SH], F32)
            nc.sync.dma_start(sb_x[:], xt[:])

            # weights split across both HWDGE rings (SP + ACT) so the two
            # halves transfer concurrently; separate tiles give the scheduler
            # independent completion tracking.
            sb_wa = sb.tile([IN_DIM, H, OUT_DIM], F32R, tag="wa")
            nc.sync.dma_start(sb_wa[:], w[:, 0:H, :])
            sb_wb = sb.tile([IN_DIM, 10 - H, OUT_DIM], F32R, tag="wb")
            nc.scalar.dma_start(sb_wb[:], w[:, H:10, :])

            # silu(x) first: only needs x, runs on ACT while DVE reduces
            feats_s = sb.tile([IN_DIM, B_SH], F32R, tag="fs")
            nc.scalar.activation(
                feats_s[:], sb_x[:], mybir.ActivationFunctionType.Silu
            )

            # sin features, two pipelined halves of 4 grid values each.
            # ACT sin is only accurate on ~[-4.15, 4.15]; range-reduce on DVE
            # into [-pi, pi]: n = round(g*x/2pi) via the 1.5*2^23 magic-add
            # trick, r = g*x - 2pi*n.
            feats_h = []
            for h in range(2):
                gs = key[h * H : (h + 1) * H]
                t4 = sb.tile([IN_DIM, H, B_SH], F32, tag=f"t4_{h}")
                for j, g in enumerate(gs):
                    nc.vector.tensor_scalar(
                        t4[:, j, :], sb_x[:], float(g), None, mybir.AluOpType.mult
                    )
                tn = sb.tile([IN_DIM, H, B_SH], F32, tag=f"tn_{h}")
                nc.vector.tensor_scalar(
                    tn[:], t4[:], 1.0 / TWO_PI, MAGIC,
                    mybir.AluOpType.mult, mybir.AluOpType.add,
                )
                nc.vector.tensor_scalar(
                    tn[:], tn[:], MAGIC, TWO_PI,
                    mybir.AluOpType.subtract, mybir.AluOpType.mult,
                )
                nc.vector.tensor_tensor(
                    t4[:], t4[:], tn[:], mybir.AluOpType.subtract
                )
                f4 = sb.tile([IN_DIM, H, B_SH], F32R, tag=f"f4_{h}")
                nc.scalar.activation(f4[:], t4[:], mybir.ActivationFunctionType.Sin)
                feats_h.append(f4)

            # 9 accumulating matmuls: psum[b, o] += feats_j.T @ W[:, j, :]
            psum = ps.tile([B_SH, OUT_DIM], F32)
            for j in range(NFUNC):
                if j < H:
                    lhsT = feats_h[0][:, j, :]
                    rhs = sb_wa[:, j, :]
                elif j < NUM_F:
                    lhsT = feats_h[1][:, j - H, :]
                    rhs = sb_wb[:, j - H, :]
                else:
                    lhsT = feats_s[:]
                    rhs = sb_wb[:, NUM_F - H, :]
                nc.tensor.matmul(
                    psum[:], lhsT=lhsT, rhs=rhs,
                    start=(j == 0), stop=(j == NFUNC - 1),
                )

            # psum + bias row (already replicated across partitions in w)
            out_sb = sb.tile([B_SH, OUT_DIM], F32)
            nc.vector.tensor_tensor(
                out_sb[:], psum[:], sb_wb[:, 9 - H, :].bitcast(F32),
                mybir.AluOpType.add,
            )
            # output in two halves on the two HWDGE rings (overlapping the
            # ~2us completion-receipt latency)
            nc.sync.dma_start(y[:, 0 : OUT_DIM // 2], out_sb[:, 0 : OUT_DIM // 2])
            nc.scalar.dma_start(y[:, OUT_DIM // 2 :], out_sb[:, OUT_DIM // 2 :])

    nc.finalize()
    _KERNEL_CACHE[key] = nc
    return nc


def _prep_inputs(x, grid, coef, scale_sp, scale_base, bias_w):
    x = np.asarray(x, dtype=np.float32)
    coef = np.asarray(coef, dtype=np.float32)
    scale_sp = np.asarray(scale_sp, dtype=np.float32)
    scale_base = np.asarray(scale_base, dtype=np.float32)
    bias_w = np.asarray(bias_w, dtype=np.float32).reshape(1, OUT_DIM)

    # fold scales into the per-basis weight matrices, laid out [i, j, o]
    coef3 = coef.reshape(OUT_DIM, IN_DIM, NUM_F)  # [o, i, k]
    ssp = scale_sp.reshape(OUT_DIM, IN_DIM)  # [o, i]
    sba = scale_base.reshape(OUT_DIM, IN_DIM)
    w = np.zeros((IN_DIM, 10, OUT_DIM), np.float32)
    w[:, :NUM_F, :] = np.transpose(coef3 * ssp[:, :, None], (1, 2, 0))
    w[:, NUM_F, :] = sba.T
    w[:, NUM_F + 1, :] = bias_w[0][None, :]  # bias row, replicated over i

    xt = np.ascontiguousarray(x.T)  # [i, b] full

    in_maps = []
    for c in range(N_CORES):
        in_maps.append(
            {
                "xt": np.ascontiguousarray(xt[:, c * B_SH : (c + 1) * B_SH]),
                "w": w,
            }
        )
    return in_maps


def _run(inputs, trace=False):
    nc = _build(np.asarray(inputs["grid"], dtype=np.float32))
    in_maps = _prep_inputs(
        inputs["x"],
        inputs["grid"],
        inputs["coef"],
        inputs["scale_sp"],
        inputs["scale_base"],
        inputs["bias_w"],
    )
    res = run_bass_kernel_spmd(nc, in_maps, list(range(N_CORES)), trace=trace)
    y = np.concatenate([res.results[c]["y"] for c in range(N_CORES)], axis=0)
    return y.astype(np.float32), res


def kernel(**inputs):
    y, _ = _run(inputs, trace=False)
    return y
